# revision 26
# baseline (speedup 1.0000x reference)
"""Multi-head attention (B=2, P=2048, DIM=1024, H=16, d=64) on 8 trn2 cores.

Sharding: batches processed SEQUENTIALLY; for each batch, all 8 cores work
on it together — core c handles heads {2c, 2c+1} over the full sequence,
and owns output q-slice [256c, 256c+256) of both batches.

Per core, per batch:
  - QKV projection for its 2 heads in transposed layout (Q^T, K^T: [128
    (head-parity, d), seq]) off bf16 x^T; V ([seq, (parity, d)]) with a ones
    column (denominator trick).
  - Attention in S^T orientation, one k-chunk step at a time: the two heads'
    S^T matmuls (contraction d=64) go to PE row-tiles (0,0)/(64,0) and run
    CONCURRENTLY on hw; exp on ScalarE over both heads' tiles at once
    ([128, 2, 512], scale 1/8 folded); AV accumulates per head with V
    augmented by the ones column (denominator lands in PSUM row 64).
    Normalize with DVE reciprocal + gpsimd partition_broadcast + DVE mul.
  - One AllToAll over all 8 cores per batch exchanges O^T q-slices in bf16
    ([8 slots, 2 heads, 64, 256]); every slot is useful (no zero padding).
  - Output projection over the gathered [1024 x 256] O^T with dense bf16
    W_proj (identical on every core) + bias -> [256, 1024] out slice.

Pipelining: batch-1 QKV rides in batch-0 rounds' PE slack; batch-0's A2A and
projection overlap batch-1 rounds. Across repeat passes the NEXT pass's xt
loads + QKV startup run inside the current pass's tail A2A window (og DMA for
batch 1 goes out on the Activation hwdge queue so those xt loads don't
head-of-line block behind it on SP), so in steady state only rounds remain
on the critical path.
"""

import contextlib as _contextlib
import sys

sys.path.insert(0, "/opt/trn_rl_repo")

import numpy as np
import concourse.bass as bass
import concourse.tile as tile
import concourse.mybir as mybir
from concourse import bacc
from concourse.bass import ts
from concourse.bass_utils import run_bass_kernel_spmd

FP = mybir.dt.float32
BF = mybir.dt.bfloat16
N_CORES = 8
B, P, DIM, H, D = 2, 2048, 1024, 16, 64
DHC = 2 * D  # dh per core = 128 (2 heads)
QS = P // N_CORES  # per-core output q-slice = 256
NQ = P // 512  # 4 q-chunks of 512
NK = P // 128  # 16 k-chunks of 128
ND = DIM // 128  # 8 dim-chunks
MM_DT = mybir.dt.float32r  # S matmul operand dtype (full PE rate at >=256)
EX_DT = mybir.dt.bfloat16  # exp output / AV moving operand dtype
PEND_LAG = 8  # AV flush lag (pend entries = 4 steps; keeps AV deps ancient)

_CACHE = {}


def _build(repeat=1, fake_cc=False, warmup=12, av8=0, pair_s=1, bare=0, lag=PEND_LAG, esb=6, avpb=3, flpb=1, kq1p=0, vb1h=0, pj1f=0, exp2=0, nodrain=1, fastpro=3, bpop=1, stpb=2, deep=0):
    if deep:
        # depth-3 S-tile pool so exp(k)'s input is ready before exp(k-1)
        # finishes (Act streams back-to-back); PSUM budget forces avp=2 and
        # no filler pool (QKV all in prologue, proj post-rounds via avp)
        stpb, avpb, flpb, kq1p = 3, 2, 0, 1
    nc = bacc.Bacc(
        "TRN2",
        target_bir_lowering=False,
        debug=False,
        enable_asserts=False,
        num_devices=N_CORES,
    )
    xt = nc.dram_tensor("xt", [B, DIM, P], BF, kind="ExternalInput").ap()
    wq = nc.dram_tensor("wq", [DIM, DHC], BF, kind="ExternalInput").ap()
    wk = nc.dram_tensor("wk", [DIM, DHC], BF, kind="ExternalInput").ap()
    wv = nc.dram_tensor("wv", [DIM, DHC], BF, kind="ExternalInput").ap()
    wp = nc.dram_tensor("wp", [DIM, DIM], BF, kind="ExternalInput").ap()
    bias = nc.dram_tensor("bias", [128, DIM], FP, kind="ExternalInput").ap()
    out = nc.dram_tensor("out", [B, QS, DIM], FP, kind="ExternalOutput").ap()

    with tile.TileContext(nc) as tc:
        with (
            tc.tile_pool(name="s1", bufs=1) as s1,
            tc.tile_pool(name="dram", bufs=1, space="DRAM") as dram,
            tc.tile_pool(name="es", bufs=esb) as es,
            tc.tile_pool(name="wk2", bufs=2) as wk2,
            tc.tile_pool(name="stp", bufs=stpb, space="PSUM") as stp,
            tc.tile_pool(name="avp", bufs=avpb, space="PSUM") as avp,
            (
                tc.tile_pool(name="flp", bufs=flpb, space="PSUM")
                if flpb
                else _contextlib.nullcontext(None)
            ) as flp,
        ):
            import collections as _c

            xt_s = s1.tile([128, B, ND, P], BF)
            wq_s = s1.tile([128, ND, DHC], BF)
            wk_s = s1.tile([128, ND, DHC], BF)
            wv_s = s1.tile([128, ND, DHC], BF)
            qt_s = s1.tile([128, B, P], MM_DT)
            kt_s = s1.tile([128, B, NK, 128], MM_DT)
            if av8:
                # 80 = D+1 padded so the Ko-dim stride is a multiple of 16
                # (dual-fp8 ldweights ISA restriction); cols 65-79 zeroed
                v_s = s1.tile([128, B, NK // 2, 2, 2, 80], mybir.dt.float8e4)
            else:
                v_s = s1.tile([128, B, NK, 2, D + 1], EX_DT)
            wp_s = s1.tile([128, ND, DIM], BF)
            og_s = s1.tile([128, B, ND, QS], BF)
            obuf = s1.tile([128, B, 4, 512], FP)
            bias_s = s1.tile([128, DIM], FP)

            nc.sync.dma_start(wq_s[:], wq.rearrange("(c p) n -> p c n", p=128))
            nc.sync.dma_start(wk_s[:], wk.rearrange("(c p) n -> p c n", p=128))
            nc.sync.dma_start(wv_s[:], wv.rearrange("(c p) n -> p c n", p=128))
            if av8:
                nc.vector.memset(v_s[:, :, :, :, :, D : D + 1], 1.0)
                nc.vector.memset(v_s[:, :, :, :, :, D + 1 : 80], 0.0)
            else:
                nc.vector.memset(v_s[:, :, :, :, D : D + 1], 1.0)

            cc_in = [
                dram.tile([N_CORES, 2, D, QS], BF, name=f"cci{b}") for b in range(B)
            ]
            cc_out = [
                dram.tile([N_CORES, 2, D, QS], BF, name=f"cco{b}") for b in range(B)
            ]

            # ---- allocators: startup/filler chains borrow PSUM space ------
            def st_alloc():
                # startup chains ride the S-tile rotation (2 banks each)
                return stp.tile([128, 2, 512], FP, tag="st", name="st")[:, 0, :]

            def fl_alloc():
                if deep:
                    # only legal when avp has a free slot (post-rounds proj /
                    # last-pass warmup); never used as an in-round filler.
                    # shares the "av" tag so no extra PSUM is reserved
                    return avp.tile([128, 512], FP, tag="av", name="fl")
                return flp.tile([128, 512], FP, tag="fl", name="fl")

            # ---- QKV / proj chain builders --------------------------------
            def k_chunk(alloc, b, qc):
                psk = alloc()
                for dc in range(ND):
                    nc.tensor.matmul(
                        psk[:],
                        wk_s[:, dc, :],
                        xt_s[:, b, dc, ts(qc, 512)],
                        start=(dc == 0),
                        stop=(dc == ND - 1),
                    )
                nc.vector.tensor_copy(
                    out=kt_s[:, b, 4 * qc : 4 * qc + 4, :],
                    in_=psk[:].rearrange("p (a k) -> p a k", k=128),
                )

            def q_chunk(alloc, b, qc):
                psq = alloc()
                for dc in range(ND):
                    nc.tensor.matmul(
                        psq[:],
                        wq_s[:, dc, :],
                        xt_s[:, b, dc, ts(qc, 512)],
                        start=(dc == 0),
                        stop=(dc == ND - 1),
                    )
                nc.vector.tensor_copy(out=qt_s[:, b, ts(qc, 512)], in_=psq[:])

            def v_chunk(alloc, b, sc):
                psv = alloc()
                for dc in range(ND):
                    nc.tensor.matmul(
                        psv[:, 0:DHC],
                        xt_s[:, b, dc, ts(sc, 128)],
                        wv_s[:, dc, :],
                        start=(dc == 0),
                        stop=(dc == ND - 1),
                    )
                dst = (
                    v_s[:, b, sc // 2, :, sc % 2, 0:D]
                    if av8
                    else v_s[:, b, sc, :, 0:D]
                )
                nc.vector.tensor_copy(
                    out=dst,
                    in_=psv[:, 0:DHC].rearrange("p (h d) -> p h d", d=D),
                )

            def load_xt(b, qc):
                for dc in range(ND):
                    nc.sync.dma_start(
                        xt_s[:, b, dc, ts(qc, 512)],
                        xt[b, ts(dc, 128), ts(qc, 512)],
                    )

            def prologue(first):
                if fastpro >= 3 and not first:
                    # v2 pipeline: everything (loads + b0 QKV) rode as
                    # fillers of the previous rounds(1); nothing to do here
                    return
                for qc in range(NQ):
                    load_xt(0, qc)
                for qc in range(NQ):
                    load_xt(1, qc)
                if first:
                    # wp/bias aren't needed until projection; keep them
                    # behind the xt loads so they don't delay startup
                    nc.sync.dma_start(bias_s[:], bias[:])
                    nc.sync.dma_start(
                        wp_s[:], wp.rearrange("(c p) n -> p c n", p=128)
                    )
                if fastpro and fastpro < 3 and not first:
                    # minimal serial head: kt[0:4], qt[qc0], v[kc 0..7] (b0);
                    # everything else rides as deadline-ordered round fillers
                    k_chunk(st_alloc, 0, 0)
                    q_chunk(st_alloc, 0, 0)
                    nv = 8 if fastpro >= 2 else NK
                    for sc in range(nv):
                        v_chunk(st_alloc, 0, sc)
                    return
                for qc in range(NQ):
                    k_chunk(st_alloc, 0, qc)
                for sc in range(NK):
                    v_chunk(st_alloc, 0, sc)
                q_chunk(st_alloc, 0, 0)
                for sc in range(NK // 2 if vb1h else NK):
                    v_chunk(st_alloc, 1, sc)
                if kq1p:
                    for qc in range(NQ):
                        k_chunk(st_alloc, 1, qc)
                    for qc in range(NQ):
                        q_chunk(st_alloc, 1, qc)
                if deep:
                    for qc in range(1, NQ):
                        q_chunk(st_alloc, 0, qc)

            # ---- attention round machinery --------------------------------
            pend = _c.deque()  # (b, p, av, ex, kc, qc|None)
            ex_cur = [None]  # av8: ex tile spanning a kc pair

            def emit_tail(b, p, qc, av):
                rec = wk2.tile([1, 512], FP, tag="rec", name="rec")
                nc.vector.reciprocal(rec[:], av[D : D + 1, :])
                bc = wk2.tile([D, 512], FP, tag="bc", name="bc")
                nc.gpsimd.partition_broadcast(bc[:], rec[:])
                om = wk2.tile([D, 512], BF, tag="om", name="om")
                nc.vector.tensor_mul(om[:], av[0:D, :], bc[:])
                if bare:
                    return
                # q-chunk qc covers A2A slots 2qc, 2qc+1 (256 each); one DMA
                # per slot keeps the SBUF source partition-first
                for s2 in range(2):
                    nc.sync.dma_start(
                        cc_in[b][2 * qc + s2, p, :, :],
                        om[:, ts(s2, QS)],
                    )

            def flush_one():
                b_, p_, av_, ex_, kc_, qc_ = pend.popleft()
                if av8:
                    nc.tensor.matmul(
                        av_[0:80, :],
                        v_s[:, b_, kc_, p_, :, :],
                        ex_[:, p_, :, :],
                        start=(kc_ == 0),
                        stop=(kc_ == NK // 2 - 1),
                        perf_mode=mybir.MatmulPerfMode.DoubleRow,
                        skip_group_check=True,
                    )
                else:
                    nc.tensor.matmul(
                        av_[0 : D + 1, :],
                        v_s[:, b_, kc_, p_, :],
                        ex_[:, p_, :],
                        start=(kc_ == 0),
                        stop=(kc_ == NK - 1),
                        skip_group_check=True,
                    )
                if qc_ is not None:
                    emit_tail(b_, p_, qc_, av_)

            def emit_cc(b):
                if fake_cc:
                    nc.sync.dma_start(cc_out[b][:], cc_in[b][:])
                else:
                    nc.gpsimd.collective_compute(
                        "AllToAll",
                        mybir.AluOpType.bypass,
                        replica_groups=[list(range(N_CORES))],
                        ins=[cc_in[b].opt()],
                        outs=[cc_out[b].opt()],
                    )

            def og_dma(b, eng):
                eng.dma_start(
                    og_s[:, b, :, :],
                    cc_out[b].rearrange("s h p n -> (h p) s n"),
                )

            def proj_u(b, u):
                oc, sc = divmod(u, 2)
                pso = fl_alloc()
                for c in range(ND):
                    nc.tensor.matmul(
                        pso[:],
                        og_s[:, b, c, ts(sc, 128)],
                        wp_s[:, c, ts(oc, 512)],
                        start=(c == 0),
                        stop=(c == ND - 1),
                    )
                nc.vector.tensor_add(
                    obuf[:, b, u, :], pso[:], bias_s[:, ts(oc, 512)]
                )
                nc.sync.dma_start(
                    out[b, ts(sc, 128), ts(oc, 512)], obuf[:, b, u, :]
                )

            def rounds(b, fillers):
                fq = _c.deque(fillers)
                for qc in range(NQ):
                    av = [
                        avp.tile([128, 512], FP, tag="av", name=f"av{p}")
                        for p in (0, 1)
                    ]
                    for kc in range(NK):
                        st = stp.tile([128, 2, 512], FP, tag="st", name="st")
                        for p in (0, 1):
                            nc.tensor.matmul(
                                st[:, p, :],
                                kt_s[ts(p, D), b, kc, :],
                                qt_s[ts(p, D), b, ts(qc, 512)],
                                start=True,
                                stop=True,
                            )
                            if not pair_s and p == 0 and pend:
                                # control: a full-row AV between the two
                                # row-tile S matmuls kills hw concurrency
                                flush_one()
                        if av8:
                            if kc % 2 == 0:
                                ex_cur[0] = es.tile(
                                    [128, 2, 2, 512],
                                    mybir.dt.float8e4,
                                    tag="ex",
                                    name="ex",
                                )
                            ex = ex_cur[0]
                            nc.scalar.activation(
                                out=ex[:, :, kc % 2, :],
                                in_=st[:],
                                func=mybir.ActivationFunctionType.Exp,
                                scale=float(D) ** -0.5,
                            )
                            if kc % 2 == 1:
                                for p in (0, 1):
                                    pend.append(
                                        (
                                            b,
                                            p,
                                            av[p],
                                            ex,
                                            kc // 2,
                                            qc if kc == NK - 1 else None,
                                        )
                                    )
                        else:
                            ex = es.tile([128, 2, 512], EX_DT, tag="ex", name="ex")
                            nc.scalar.activation(
                                out=ex[:],
                                in_=st[:],
                                func=mybir.ActivationFunctionType.Exp,
                                scale=float(D) ** -0.5,
                            )
                            if exp2:
                                ex_d = es.tile(
                                    [128, 2, 512], EX_DT, tag="ex", name="exd"
                                )
                                nc.scalar.activation(
                                    out=ex_d[:],
                                    in_=st[:],
                                    func=mybir.ActivationFunctionType.Exp,
                                    scale=float(D) ** -0.5,
                                )
                            for p in (0, 1):
                                pend.append(
                                    (
                                        b,
                                        p,
                                        av[p],
                                        ex,
                                        kc,
                                        qc if kc == NK - 1 else None,
                                    )
                                )
                        # bpop: pop down to 4 at the qc boundary so the last
                        # flushes (and emit_tail) land promptly, freeing av
                        # tiles for the next qc's rotation
                        thr = min(lag, 4) if (bpop and kc == NK - 1) else lag
                        while len(pend) > thr:
                            flush_one()
                        if fq:
                            f = fq.popleft()
                            if f is not None:
                                f()
                    if not nodrain:
                        # drain round so avp (bufs=3) can rotate
                        while pend:
                            flush_one()
                while pend:
                    flush_one()
                while fq:
                    f = fq.popleft()
                    if f is not None:
                        f()

            # ---- schedule ------------------------------------------------
            deferred_proj = [[]]

            def one_body(last):
                if deep:
                    rounds(0, [])
                    emit_cc(0)
                    fillers_b1 = [None] * 50 + [lambda: og_dma(0, nc.sync)]
                    rounds(1, fillers_b1)
                    emit_cc(1)
                    og_dma(1, nc.scalar)
                    if not last:
                        prologue(first=False)
                    for u in range(4):
                        proj_u(0, u)
                    for u in range(4):
                        proj_u(1, u)
                    return
                if fastpro >= 3:
                    # v2 software pipeline: this pass's b0 QKV was computed
                    # in the previous rounds(1); rounds(0) fillers complete
                    # b0 q-chunks + all of b1 QKV; the previous pass's
                    # deferred proj(1) rides at the end (og(1) certain by
                    # then). rounds(1) fillers carry the NEXT pass's xt
                    # loads + b0 QKV, og(0) + proj(0).
                    fillers_b0 = (
                        [lambda qc=qc: q_chunk(fl_alloc, 0, qc) for qc in range(1, NQ)]
                        + [lambda qc=qc: k_chunk(fl_alloc, 1, qc) for qc in range(NQ)]
                        + [lambda qc=qc: q_chunk(fl_alloc, 1, qc) for qc in range(NQ)]
                        + [lambda sc=sc: v_chunk(fl_alloc, 1, sc) for sc in range(NK)]
                        + deferred_proj[0]
                    )
                    deferred_proj[0] = []
                elif fastpro:
                    # deadline-ordered completion of this pass's b0 QKV
                    # (deferred from the reduced prologue), then b1 QKV.
                    # k_chunk(b0,j) must finish by step 4j; v_chunk(b0,sc)
                    # by step sc+2; q_chunk(b0,qc) by step 16*qc.
                    kb0 = [lambda qc=qc: k_chunk(fl_alloc, 0, qc) for qc in range(1, NQ)]
                    qb0 = [lambda qc=qc: q_chunk(fl_alloc, 0, qc) for qc in range(1, NQ)]
                    if fastpro >= 2:
                        vb0 = [lambda sc=sc: v_chunk(fl_alloc, 0, sc) for sc in range(8, NK)]
                        head = [
                            kb0[0], kb0[1], vb0[0], vb0[1], kb0[2], vb0[2],
                            vb0[3], qb0[0], vb0[4], vb0[5], vb0[6], vb0[7],
                            qb0[1], qb0[2],
                        ]
                    else:
                        head = [kb0[0], kb0[1], kb0[2], qb0[0], qb0[1], qb0[2]]
                    fillers_b0 = (
                        head
                        + deferred_proj[0]
                        + [lambda qc=qc: k_chunk(fl_alloc, 1, qc) for qc in range(NQ)]
                        + [lambda qc=qc: q_chunk(fl_alloc, 1, qc) for qc in range(NQ)]
                        + [lambda sc=sc: v_chunk(fl_alloc, 1, sc) for sc in range(NK)]
                    )
                    deferred_proj[0] = []
                else:
                    fillers_b0 = [
                        lambda qc=qc: q_chunk(fl_alloc, 0, qc) for qc in range(1, NQ)
                    ] + (
                        []
                        if kq1p
                        else [lambda qc=qc: k_chunk(fl_alloc, 1, qc) for qc in range(NQ)]
                        + [lambda qc=qc: q_chunk(fl_alloc, 1, qc) for qc in range(NQ)]
                    ) + (
                        [lambda sc=sc: v_chunk(fl_alloc, 1, sc) for sc in range(NK // 2, NK)]
                        if vb1h
                        else []
                    )
                if bare >= 2:
                    fillers_b0 = []
                fillers_b0 = deferred_proj[0] + fillers_b0
                deferred_proj[0] = []
                rounds(0, fillers_b0)
                if bare:
                    rounds(1, [])
                    nc.sync.dma_start(out[0, 0:128, 0:512], qt_s[:, 0, 0:512].bitcast(FP))
                    return
                emit_cc(0)

                # og0 enters the SP queue mid-rounds-b1 (step 36): A2A(0) is
                # done by then, so earlier rounds' om DMAs don't sit behind
                # its semaphore wait; proj-b0 rides later still
                # sequencers dispatch ~16 steps ahead; A2A(0)-dependent work
                # must sit late enough that its queue reaches it only after
                # the collective completes
                if fastpro >= 3 and not last:
                    nxt = (
                        [lambda b=b, qc=qc: load_xt(b, qc) for b in range(B) for qc in range(NQ)]
                        + [None] * 4
                        + [lambda qc=qc: k_chunk(fl_alloc, 0, qc) for qc in range(NQ)]
                        + [lambda sc=sc: v_chunk(fl_alloc, 0, sc) for sc in range(NK)]
                        + [lambda: q_chunk(fl_alloc, 0, 0)]
                    )
                    fillers_b1 = (
                        nxt
                        + [None] * (50 - len(nxt))
                        + [lambda: og_dma(0, nc.sync)]
                        + [None] * 9
                        + [lambda u=u: proj_u(0, u) for u in range(4)]
                    )
                else:
                    fillers_b1 = (
                        [None] * 50
                        + [lambda: og_dma(0, nc.sync)]
                        + [None] * 9
                        + [lambda u=u: proj_u(0, u) for u in range(4)]
                    )
                rounds(1, fillers_b1)
                emit_cc(1)
                if fastpro >= 3:
                    # SP is clean here (next pass's xt loads already issued
                    # inside rounds(1)), so og(1) can ride SP without
                    # head-of-line blocking anything urgent
                    og_dma(1, nc.sync)
                else:
                    # ACT hwdge queue: the next pass's xt loads on SP must
                    # not head-of-line block behind this A2A-dependent DMA
                    og_dma(1, nc.scalar)

                if last:
                    # keep the PE clock warm through the tail A2A window so
                    # proj-b1 doesn't run at the cold p-state
                    for w in range(warmup):
                        psw = fl_alloc()
                        for c in range(ND):
                            nc.tensor.matmul(
                                psw[:],
                                og_s[:, 0, c, 0:128],
                                wp_s[:, c, 0:512],
                                start=(c == 0),
                                stop=(c == ND - 1),
                            )
                else:
                    # next pass's prologue fills the tail A2A window with
                    # real work (and keeps the PE clock warm)
                    prologue(first=False)
                if (pj1f or fastpro >= 3) and not last:
                    # og(1) is ready well before the next rounds-b0 begins;
                    # running proj-b1 as its early fillers removes ~7us of
                    # serial tail per pass
                    deferred_proj[0] = [lambda u=u: proj_u(1, u) for u in range(4)]
                else:
                    for u in range(4):
                        proj_u(1, u)

            prologue(first=True)
            for rep in range(repeat):
                one_body(last=(rep == repeat - 1) or bare)
                if bare and rep + 1 < repeat:
                    prologue(first=False)

    nc.compile()
    return nc


def _prep_inputs(x, W_qkv, W_proj, b_proj):
    """Host-side sharding: per-core input dicts."""
    import ml_dtypes

    bf16 = np.dtype(ml_dtypes.bfloat16)
    x = np.asarray(x, dtype=np.float32)
    W_qkv = np.asarray(W_qkv, dtype=np.float32)
    W_proj = np.asarray(W_proj, dtype=np.float32)
    b_proj = np.asarray(b_proj, dtype=np.float32)

    # xt [B, DIM, P] bf16 — identical on every core
    xt = np.ascontiguousarray(x.transpose(0, 2, 1)).astype(bf16)
    # wp row 128s + 64p + i = W_proj[64*(2s+p) + i] — exactly natural order
    wp = np.ascontiguousarray(W_proj).astype(bf16)
    bias_b = np.ascontiguousarray(np.broadcast_to(b_proj[None, :], (128, DIM)))

    in_maps = []
    for c in range(N_CORES):
        h0 = 2 * c
        cols = slice(D * h0, D * (h0 + 2))
        in_maps.append(
            {
                "xt": xt,
                "wq": np.ascontiguousarray(W_qkv[:, 0 * DIM :][:, cols]).astype(bf16),
                "wk": np.ascontiguousarray(W_qkv[:, 1 * DIM :][:, cols]).astype(bf16),
                "wv": np.ascontiguousarray(W_qkv[:, 2 * DIM :][:, cols]).astype(bf16),
                "wp": wp,
                "bias": bias_b,
            }
        )
    return in_maps


def kernel(x, W_qkv, W_proj, b_proj, _trace=False, _tmpdir=None):
    if "nc" not in _CACHE:
        _CACHE["nc"] = _build()
    nc = _CACHE["nc"]
    in_maps = _prep_inputs(x, W_qkv, W_proj, b_proj)
    res = run_bass_kernel_spmd(
        nc,
        in_maps,
        core_ids=list(range(N_CORES)),
        trace=_trace,
        tmpdir=_tmpdir,
        stitch_traces=False,
    )
    _CACHE["last_results"] = res
    full = np.empty((B, P, DIM), dtype=np.float32)
    for c in range(N_CORES):
        o = np.asarray(res.results[c]["out"])
        for b in range(B):
            full[b, QS * c : QS * (c + 1), :] = o[b]
    return full



# revision 62
# speedup vs baseline: 1.2350x; 1.2350x over previous
"""Multi-head attention (B=2, P=2048, DIM=1024, H=16, d=64) on 8 trn2 cores.

Sharding: batches processed SEQUENTIALLY; for each batch, all 8 cores work
on it together — core c handles heads {2c, 2c+1} over the full sequence,
and owns output q-slice [256c, 256c+256) of both batches.

Per core, per batch:
  - QKV projection for its 2 heads in transposed layout (Q^T, K^T: [128
    (head-parity, d), seq]) off bf16 x^T; V ([seq, (parity, d)]) with a ones
    column (denominator trick).
  - Attention in S^T orientation, one k-chunk step at a time: the two heads'
    S^T matmuls (contraction d=64) go to PE row-tiles (0,0)/(64,0) and run
    CONCURRENTLY on hw; exp on ScalarE over both heads' tiles at once
    ([128, 2, 512], scale 1/8 folded); AV accumulates per head with V
    augmented by the ones column (denominator lands in PSUM row 64).
    Normalize with DVE reciprocal + gpsimd partition_broadcast + DVE mul.
  - One AllToAll over all 8 cores per batch exchanges O^T q-slices in bf16
    ([8 slots, 2 heads, 64, 256]); every slot is useful (no zero padding).
  - Output projection over the gathered [1024 x 256] O^T with dense bf16
    W_proj (identical on every core) + bias -> [256, 1024] out slice.

Pipelining (fastpro=3 software pipeline): batch-1 QKV + batch-0 q-chunks ride
as rounds-b0 fillers; rounds-b1 fillers carry the NEXT pass's xt loads + its
full b0 QKV, plus og(0)+proj(0); proj(1) is deferred into the next pass's
rounds-b0 fillers. In steady state the pass is just rounds(0)+rounds(1) —
prologue, collectives, og and proj are fully hidden (bare rounds measure the
same as the full pass). AV flushes run at lag=8 (deps 4 steps old, so the
pend AV matmuls never sit on the S->exp critical chain) with a pop-to-4 at
each qc boundary (bpop) so emit_tail lands promptly and the avp pool
rotates; pend also flows across qc boundaries (nodrain).
"""

import contextlib as _contextlib
import sys

sys.path.insert(0, "/opt/trn_rl_repo")

import numpy as np
import concourse.bass as bass
import concourse.tile as tile
import concourse.mybir as mybir
from concourse import bacc
from concourse.bass import ts
from concourse.bass_utils import run_bass_kernel_spmd

FP = mybir.dt.float32
BF = mybir.dt.bfloat16
N_CORES = 8
B, P, DIM, H, D = 2, 2048, 1024, 16, 64
DHC = 2 * D  # dh per core = 128 (2 heads)
QS = P // N_CORES  # per-core output q-slice = 256
NQ = P // 512  # 4 q-chunks of 512
NK = P // 128  # 16 k-chunks of 128
ND = DIM // 128  # 8 dim-chunks
MM_DT = mybir.dt.float32r  # S operand dtype when bfmm=0; default is bf16
# (bfmm=1): same PE rate but half the xbus/SBUF stream, ~19us/pass faster
EX_DT = mybir.dt.bfloat16  # exp output / AV moving operand dtype
PEND_LAG = 8  # AV flush lag (pend entries = 4 steps; keeps AV deps ancient)

_CACHE = {}


def _build(repeat=1, fake_cc=False, warmup=12, av8=0, pair_s=1, bare=0, lag=PEND_LAG, esb=6, avpb=3, flpb=1, kq1p=0, vb1h=0, pj1f=0, exp2=0, nodrain=1, fastpro=3, bpop=1, stpb=2, deep=0, bfmm=1, flow=0, wk2b=6, ldc=0, vt=0, et=0):
    if deep:
        # depth-3 S-tile pool so exp(k)'s input is ready before exp(k-1)
        # finishes (Act streams back-to-back); PSUM budget forces avp=2 and
        # no filler pool (QKV all in prologue, proj post-rounds via avp)
        stpb, avpb, flpb, kq1p = 3, 2, 0, 1
    nc = bacc.Bacc(
        "TRN2",
        target_bir_lowering=False,
        debug=False,
        enable_asserts=False,
        num_devices=N_CORES,
    )
    xt = nc.dram_tensor("xt", [B, DIM, P], BF, kind="ExternalInput").ap()
    wq = nc.dram_tensor("wq", [DIM, DHC], BF, kind="ExternalInput").ap()
    wk = nc.dram_tensor("wk", [DIM, DHC], BF, kind="ExternalInput").ap()
    wv = nc.dram_tensor("wv", [DIM, DHC], BF, kind="ExternalInput").ap()
    wp = nc.dram_tensor("wp", [DIM, DIM], BF, kind="ExternalInput").ap()
    bias = nc.dram_tensor("bias", [128, DIM], FP, kind="ExternalInput").ap()
    out = nc.dram_tensor("out", [B, QS, DIM], FP, kind="ExternalOutput").ap()

    with tile.TileContext(nc) as tc:
        with (
            tc.tile_pool(name="s1", bufs=1) as s1,
            tc.tile_pool(name="dram", bufs=1, space="DRAM") as dram,
            tc.tile_pool(name="es", bufs=esb) as es,
            tc.tile_pool(name="wk2", bufs=wk2b) as wk2,
            tc.tile_pool(name="stp", bufs=stpb, space="PSUM") as stp,
            tc.tile_pool(name="avp", bufs=avpb, space="PSUM") as avp,
            (
                tc.tile_pool(name="flp", bufs=flpb, space="PSUM")
                if flpb
                else _contextlib.nullcontext(None)
            ) as flp,
        ):
            import collections as _c

            xt_s = s1.tile([128, B, ND, P], BF)
            wq_s = s1.tile([128, ND, DHC], BF)
            wk_s = s1.tile([128, ND, DHC], BF)
            wv_s = s1.tile([128, ND, DHC], BF)
            qt_s = s1.tile([128, B, P], BF if bfmm else MM_DT)
            kt_s = s1.tile([128, B, NK, 128], BF if bfmm else MM_DT)
            if av8:
                # 80 = D+1 padded so the Ko-dim stride is a multiple of 16
                # (dual-fp8 ldweights ISA restriction); cols 65-79 zeroed
                v_s = s1.tile([128, B, NK // 2, 2, 2, 80], mybir.dt.float8e4)
            else:
                v_s = s1.tile([128, B, NK, 2, D + 1], EX_DT)
            wp_s = s1.tile([128, ND, DIM], BF)
            if vt:
                # V^T staging: [dh, seq] per (b, qc); round-trips through
                # DRAM so the xbar transpose reads a DRAM source. Each
                # (b, kc, p) gets its OWN full-tile xbar dest (HW repro:
                # nonzero-offset transpose DESTS corrupt; offset sources +
                # full-tile dests are bit-exact). Source rows per head are
                # [64 d | ones | 15 junk] so col 64 of the dest is the
                # denominator ones column; cols 65-79 are junk, never read.
                vt_sb = s1.tile([128, B, NQ, 512], BF)
                ones_row = s1.tile([1, P], BF)
                # one [128, 4 kc, 80] full tile per (b, qc, p): the 3D
                # offset-0 dest transposes kc-major (HW-verified bit-exact),
                # so vtiles[(b,qc,p)][part, kc, col] = V^T[col, kc*128+part]
                vtiles = {
                    (b, qc, p): s1.tile([128, 4, 80], BF, name=f"vtl{b}_{qc}_{p}")
                    for b in range(B)
                    for qc in range(NQ)
                    for p in (0, 1)
                }
            og_s = s1.tile([128, B, ND, QS], BF)
            obuf = s1.tile([128, B, 4, 512], FP)
            bias_s = s1.tile([128, DIM], FP)

            nc.sync.dma_start(wq_s[:], wq.rearrange("(c p) n -> p c n", p=128))
            nc.sync.dma_start(wk_s[:], wk.rearrange("(c p) n -> p c n", p=128))
            nc.sync.dma_start(wv_s[:], wv.rearrange("(c p) n -> p c n", p=128))
            if av8:
                nc.vector.memset(v_s[:, :, :, :, :, D : D + 1], 1.0)
                nc.vector.memset(v_s[:, :, :, :, :, D + 1 : 80], 0.0)
            else:
                nc.vector.memset(v_s[:, :, :, :, D : D + 1], 1.0)

            if vt:
                vt_dram = dram.tile([B, 160, P], BF, name="vtd")
                nc.vector.memset(ones_row[:], 1.0)
                for b in range(B):
                    for p in (0, 1):
                        nc.sync.dma_start(
                            vt_dram[b, 80 * p + D : 80 * p + D + 1, :],
                            ones_row[:],
                        )
            cc_in = [
                dram.tile([N_CORES, 2, D, QS], BF, name=f"cci{b}") for b in range(B)
            ]
            cc_out = [
                dram.tile([N_CORES, 2, D, QS], BF, name=f"cco{b}") for b in range(B)
            ]

            # ---- allocators: startup/filler chains borrow PSUM space ------
            def st_alloc():
                # startup chains ride the S-tile rotation (2 banks each)
                return stp.tile([128, 2, 512], FP, tag="st", name="st")[:, 0, :]

            def fl_alloc():
                if deep:
                    # only legal when avp has a free slot (post-rounds proj /
                    # last-pass warmup); never used as an in-round filler.
                    # shares the "av" tag so no extra PSUM is reserved
                    return avp.tile([128, 512], FP, tag="av", name="fl")
                return flp.tile([128, 512], FP, tag="fl", name="fl")

            # ---- QKV / proj chain builders --------------------------------
            def k_chunk(alloc, b, qc):
                psk = alloc()
                for dc in range(ND):
                    nc.tensor.matmul(
                        psk[:],
                        wk_s[:, dc, :],
                        xt_s[:, b, dc, ts(qc, 512)],
                        start=(dc == 0),
                        stop=(dc == ND - 1),
                    )
                nc.vector.tensor_copy(
                    out=kt_s[:, b, 4 * qc : 4 * qc + 4, :],
                    in_=psk[:].rearrange("p (a k) -> p a k", k=128),
                )

            def q_chunk(alloc, b, qc):
                psq = alloc()
                for dc in range(ND):
                    nc.tensor.matmul(
                        psq[:],
                        wq_s[:, dc, :],
                        xt_s[:, b, dc, ts(qc, 512)],
                        start=(dc == 0),
                        stop=(dc == ND - 1),
                    )
                nc.vector.tensor_copy(out=qt_s[:, b, ts(qc, 512)], in_=psq[:])

            def v_chunk(alloc, b, sc):
                psv = alloc()
                for dc in range(ND):
                    nc.tensor.matmul(
                        psv[:, 0:DHC],
                        xt_s[:, b, dc, ts(sc, 128)],
                        wv_s[:, dc, :],
                        start=(dc == 0),
                        stop=(dc == ND - 1),
                    )
                dst = (
                    v_s[:, b, sc // 2, :, sc % 2, 0:D]
                    if av8
                    else v_s[:, b, sc, :, 0:D]
                )
                nc.vector.tensor_copy(
                    out=dst,
                    in_=psv[:, 0:DHC].rearrange("p (h d) -> p h d", d=D),
                )

            def vt_chunk(alloc, b, qc):
                # V^T [dh, 512 seq] via wide moving (like k_chunk: 8 LDW +
                # 8 MM(512) instead of v_chunk's 8x(LDW + MM(128)) per 128
                # seq). Transposition into v_s [seq, dh] follows the ONLY
                # HW-validated xbar idiom (test_dma_transpose2): DRAM
                # source, 2D contiguous SBUF dest per 128-col slice. The
                # SBUF-source / 3D-strided-out form passes CoreSim but NaNs
                # on hardware.
                psv = alloc()
                for dc in range(ND):
                    nc.tensor.matmul(
                        psv[:],
                        wv_s[:, dc, :],
                        xt_s[:, b, dc, ts(qc, 512)],
                        start=(dc == 0),
                        stop=(dc == ND - 1),
                    )
                nc.vector.tensor_copy(out=vt_sb[:, b, qc, :], in_=psv[:])
                # stage-writes on the idle gpsimd SWDGE queue: cross-queue
                # from the SP transposes, so the RAW dep is sem-enforced
                for p in (0, 1):
                    nc.gpsimd.dma_start(
                        vt_dram[b, 80 * p : 80 * p + D, ts(qc, 512)],
                        vt_sb[ts(p, D), b, qc, :],
                    )
                for p in (0, 1):
                    nc.sync.dma_start_transpose(
                        vtiles[(b, qc, p)][:],
                        vt_dram[b, 80 * p : 80 * p + 80, ts(qc, 512)],
                    )

            def load_xt(b, qc):
                if ldc:
                    # one DMA per (b, qc): 8x fewer SP descriptor issues, so
                    # in-round om/cci DMAs don't queue behind the loads
                    nc.sync.dma_start(
                        xt_s[:, b, :, ts(qc, 512)],
                        xt[b].rearrange("(c p) n -> p c n", p=128)[
                            :, :, ts(qc, 512)
                        ],
                    )
                    return
                for dc in range(ND):
                    nc.sync.dma_start(
                        xt_s[:, b, dc, ts(qc, 512)],
                        xt[b, ts(dc, 128), ts(qc, 512)],
                    )

            def prologue(first):
                if fastpro >= 3 and not first:
                    # v2 pipeline: everything (loads + b0 QKV) rode as
                    # fillers of the previous rounds(1); nothing to do here
                    return
                for qc in range(NQ):
                    load_xt(0, qc)
                for qc in range(NQ):
                    load_xt(1, qc)
                if first:
                    # wp/bias aren't needed until projection; keep them
                    # behind the xt loads so they don't delay startup
                    nc.sync.dma_start(bias_s[:], bias[:])
                    nc.sync.dma_start(
                        wp_s[:], wp.rearrange("(c p) n -> p c n", p=128)
                    )
                if fastpro and fastpro < 3 and not first:
                    # minimal serial head: kt[0:4], qt[qc0], v[kc 0..7] (b0);
                    # everything else rides as deadline-ordered round fillers
                    k_chunk(st_alloc, 0, 0)
                    q_chunk(st_alloc, 0, 0)
                    nv = 8 if fastpro >= 2 else NK
                    for sc in range(nv):
                        v_chunk(st_alloc, 0, sc)
                    return
                for qc in range(NQ):
                    k_chunk(st_alloc, 0, qc)
                if vt:
                    for qc in range(NQ):
                        vt_chunk(st_alloc, 0, qc)
                else:
                    for sc in range(NK):
                        v_chunk(st_alloc, 0, sc)
                q_chunk(st_alloc, 0, 0)
                if vt:
                    for qc in range(NQ):
                        vt_chunk(st_alloc, 1, qc)
                else:
                    for sc in range(NK // 2 if vb1h else NK):
                        v_chunk(st_alloc, 1, sc)
                if kq1p:
                    for qc in range(NQ):
                        k_chunk(st_alloc, 1, qc)
                    for qc in range(NQ):
                        q_chunk(st_alloc, 1, qc)
                if deep:
                    for qc in range(1, NQ):
                        q_chunk(st_alloc, 0, qc)

            # ---- attention round machinery --------------------------------
            pend = _c.deque()  # (b, p, av, ex, kc, qc|None)
            ex_cur = [None]  # av8: ex tile spanning a kc pair

            def emit_tail(b, p, qc, av):
                rec = wk2.tile([1, 512], FP, tag="rec", name="rec")
                nc.vector.reciprocal(rec[:], av[D : D + 1, :])
                bc = wk2.tile([D, 512], FP, tag="bc", name="bc")
                nc.gpsimd.partition_broadcast(bc[:], rec[:])
                om = wk2.tile([D, 512], BF, tag="om", name="om")
                nc.vector.tensor_mul(om[:], av[0:D, :], bc[:])
                if bare:
                    return
                # q-chunk qc covers A2A slots 2qc, 2qc+1 (256 each); one DMA
                # per slot keeps the SBUF source partition-first
                for s2 in range(2):
                    nc.sync.dma_start(
                        cc_in[b][2 * qc + s2, p, :, :],
                        om[:, ts(s2, QS)],
                    )

            def flush_one():
                b_, p_, av_, ex_, kc_, qc_ = pend.popleft()
                if av8:
                    nc.tensor.matmul(
                        av_[0:80, :],
                        v_s[:, b_, kc_, p_, :, :],
                        ex_[:, p_, :, :],
                        start=(kc_ == 0),
                        stop=(kc_ == NK // 2 - 1),
                        perf_mode=mybir.MatmulPerfMode.DoubleRow,
                        skip_group_check=True,
                    )
                else:
                    nc.tensor.matmul(
                        av_[0 : D + 1, :],
                        vtiles[(b_, kc_ // 4, p_)][:, kc_ % 4, 0 : D + 1]
                        if vt
                        else v_s[:, b_, kc_, p_, :],
                        ex_[:, p_, :],
                        start=(kc_ == 0),
                        stop=(kc_ == NK - 1),
                        skip_group_check=True,
                    )
                if qc_ is not None:
                    emit_tail(b_, p_, qc_, av_)

            def emit_cc(b):
                if fake_cc:
                    nc.sync.dma_start(cc_out[b][:], cc_in[b][:])
                else:
                    nc.gpsimd.collective_compute(
                        "AllToAll",
                        mybir.AluOpType.bypass,
                        replica_groups=[list(range(N_CORES))],
                        ins=[cc_in[b].opt()],
                        outs=[cc_out[b].opt()],
                    )

            def og_dma(b, eng):
                eng.dma_start(
                    og_s[:, b, :, :],
                    cc_out[b].rearrange("s h p n -> (h p) s n"),
                )

            def proj_u(b, u):
                oc, sc = divmod(u, 2)
                pso = fl_alloc()
                for c in range(ND):
                    nc.tensor.matmul(
                        pso[:],
                        og_s[:, b, c, ts(sc, 128)],
                        wp_s[:, c, ts(oc, 512)],
                        start=(c == 0),
                        stop=(c == ND - 1),
                    )
                nc.vector.tensor_add(
                    obuf[:, b, u, :], pso[:], bias_s[:, ts(oc, 512)]
                )
                nc.sync.dma_start(
                    out[b, ts(sc, 128), ts(oc, 512)], obuf[:, b, u, :]
                )

            def rounds(b, fillers):
                fq = _c.deque(fillers)
                for qc in range(NQ):
                    av = [
                        avp.tile([128, 512], FP, tag="av", name=f"av{p}")
                        for p in (0, 1)
                    ]
                    for kc in range(NK):
                        st = stp.tile([128, 2, 512], FP, tag="st", name="st")
                        for p in (0, 1):
                            nc.tensor.matmul(
                                st[:, p, :],
                                kt_s[ts(p, D), b, kc, :],
                                qt_s[ts(p, D), b, ts(qc, 512)],
                                start=True,
                                stop=True,
                            )
                            if not pair_s and p == 0 and pend:
                                # control: a full-row AV between the two
                                # row-tile S matmuls kills hw concurrency
                                flush_one()
                        if av8:
                            if kc % 2 == 0:
                                ex_cur[0] = es.tile(
                                    [128, 2, 2, 512],
                                    mybir.dt.float8e4,
                                    tag="ex",
                                    name="ex",
                                )
                            ex = ex_cur[0]
                            nc.scalar.activation(
                                out=ex[:, :, kc % 2, :],
                                in_=st[:],
                                func=mybir.ActivationFunctionType.Exp,
                                scale=float(D) ** -0.5,
                            )
                            if kc % 2 == 1:
                                for p in (0, 1):
                                    pend.append(
                                        (
                                            b,
                                            p,
                                            av[p],
                                            ex,
                                            kc // 2,
                                            qc if kc == NK - 1 else None,
                                        )
                                    )
                        else:
                            ex = es.tile([128, 2, 512], EX_DT, tag="ex", name="ex")
                            nc.scalar.activation(
                                out=ex[:],
                                in_=st[:],
                                func=mybir.ActivationFunctionType.Exp,
                                scale=float(D) ** -0.5,
                            )
                            if exp2:
                                ex_d = es.tile(
                                    [128, 2, 512], EX_DT, tag="ex", name="exd"
                                )
                                nc.scalar.activation(
                                    out=ex_d[:],
                                    in_=st[:],
                                    func=mybir.ActivationFunctionType.Exp,
                                    scale=float(D) ** -0.5,
                                )
                            for p in (0, 1):
                                pend.append(
                                    (
                                        b,
                                        p,
                                        av[p],
                                        ex,
                                        kc,
                                        qc if kc == NK - 1 else None,
                                    )
                                )
                        # bpop: pop down at the qc boundary so the last
                        # flushes (and emit_tail) land promptly, freeing av
                        # tiles for the next qc's rotation (bpop>1 = custom
                        # threshold, bpop=1 = 4)
                        thr = (
                            min(lag, bpop if bpop > 1 else 4)
                            if (bpop and kc == NK - 1)
                            else lag
                        )
                        while len(pend) > thr:
                            flush_one()
                        if fq:
                            f = fq.popleft()
                            if f is not None:
                                f()
                    if not nodrain:
                        # drain round so avp (bufs=3) can rotate
                        while pend:
                            flush_one()
                if not flow:
                    # flow=1 lets the AV pend flow across the batch seam;
                    # the caller drains before anything that needs cc_in
                    while pend:
                        flush_one()
                while fq:
                    f = fq.popleft()
                    if f is not None:
                        f()

            # ---- schedule ------------------------------------------------
            deferred_proj = [[]]

            def one_body(last):
                if deep:
                    rounds(0, [])
                    emit_cc(0)
                    fillers_b1 = [None] * 50 + [lambda: og_dma(0, nc.sync)]
                    rounds(1, fillers_b1)
                    emit_cc(1)
                    og_dma(1, nc.scalar)
                    if not last:
                        prologue(first=False)
                    for u in range(4):
                        proj_u(0, u)
                    for u in range(4):
                        proj_u(1, u)
                    return
                if fastpro >= 3:
                    # v2 software pipeline: this pass's b0 QKV was computed
                    # in the previous rounds(1); rounds(0) fillers complete
                    # b0 q-chunks + all of b1 QKV; the previous pass's
                    # deferred proj(1) rides at the end (og(1) certain by
                    # then). rounds(1) fillers carry the NEXT pass's xt
                    # loads + b0 QKV, og(0) + proj(0).
                    vb1 = (
                        [lambda qc=qc: vt_chunk(fl_alloc, 1, qc) for qc in range(NQ)]
                        if vt
                        else [lambda sc=sc: v_chunk(fl_alloc, 1, sc) for sc in range(NK)]
                    )
                    fillers_b0 = (
                        [lambda qc=qc: q_chunk(fl_alloc, 0, qc) for qc in range(1, NQ)]
                        + [lambda qc=qc: k_chunk(fl_alloc, 1, qc) for qc in range(NQ)]
                        + [lambda qc=qc: q_chunk(fl_alloc, 1, qc) for qc in range(NQ)]
                        + vb1
                        + deferred_proj[0]
                    )
                    deferred_proj[0] = []
                elif fastpro:
                    # deadline-ordered completion of this pass's b0 QKV
                    # (deferred from the reduced prologue), then b1 QKV.
                    # k_chunk(b0,j) must finish by step 4j; v_chunk(b0,sc)
                    # by step sc+2; q_chunk(b0,qc) by step 16*qc.
                    kb0 = [lambda qc=qc: k_chunk(fl_alloc, 0, qc) for qc in range(1, NQ)]
                    qb0 = [lambda qc=qc: q_chunk(fl_alloc, 0, qc) for qc in range(1, NQ)]
                    if fastpro >= 2:
                        vb0 = [lambda sc=sc: v_chunk(fl_alloc, 0, sc) for sc in range(8, NK)]
                        head = [
                            kb0[0], kb0[1], vb0[0], vb0[1], kb0[2], vb0[2],
                            vb0[3], qb0[0], vb0[4], vb0[5], vb0[6], vb0[7],
                            qb0[1], qb0[2],
                        ]
                    else:
                        head = [kb0[0], kb0[1], kb0[2], qb0[0], qb0[1], qb0[2]]
                    fillers_b0 = (
                        head
                        + deferred_proj[0]
                        + [lambda qc=qc: k_chunk(fl_alloc, 1, qc) for qc in range(NQ)]
                        + [lambda qc=qc: q_chunk(fl_alloc, 1, qc) for qc in range(NQ)]
                        + [lambda sc=sc: v_chunk(fl_alloc, 1, sc) for sc in range(NK)]
                    )
                    deferred_proj[0] = []
                else:
                    fillers_b0 = [
                        lambda qc=qc: q_chunk(fl_alloc, 0, qc) for qc in range(1, NQ)
                    ] + (
                        []
                        if kq1p
                        else [lambda qc=qc: k_chunk(fl_alloc, 1, qc) for qc in range(NQ)]
                        + [lambda qc=qc: q_chunk(fl_alloc, 1, qc) for qc in range(NQ)]
                    ) + (
                        [lambda sc=sc: v_chunk(fl_alloc, 1, sc) for sc in range(NK // 2, NK)]
                        if vb1h
                        else []
                    )
                if bare >= 2:
                    fillers_b0 = []
                fillers_b0 = deferred_proj[0] + fillers_b0
                deferred_proj[0] = []
                rounds(0, fillers_b0)
                if bare:
                    while pend:
                        flush_one()
                    rounds(1, [])
                    while pend:
                        flush_one()
                    nc.sync.dma_start(out[0, 0:128, 0:512], qt_s[:, 0, 0:512].bitcast(FP))
                    return
                if not flow:
                    emit_cc(0)

                # og0 enters the SP queue mid-rounds-b1 (step 36): A2A(0) is
                # done by then, so earlier rounds' om DMAs don't sit behind
                # its semaphore wait; proj-b0 rides later still
                # sequencers dispatch ~16 steps ahead; A2A(0)-dependent work
                # must sit late enough that its queue reaches it only after
                # the collective completes
                cc0 = [lambda: emit_cc(0)] if flow else []
                if fastpro >= 3 and not last:
                    vb0 = (
                        [lambda qc=qc: vt_chunk(fl_alloc, 0, qc) for qc in range(NQ)]
                        if vt
                        else [lambda sc=sc: v_chunk(fl_alloc, 0, sc) for sc in range(NK)]
                    )
                    nxt = (
                        [lambda b=b, qc=qc: load_xt(b, qc) for b in range(B) for qc in range(NQ)]
                        + [None] * 4
                        + [lambda qc=qc: k_chunk(fl_alloc, 0, qc) for qc in range(NQ)]
                        + vb0
                        + [lambda: q_chunk(fl_alloc, 0, 0)]
                    )
                    # flow: pend's b0 tail drains by step ~4; A2A(0) launches
                    # from filler slot 6 (its cc_in writes are all emitted)
                    ogs = 40 if et else 50
                    fillers_b1 = (
                        nxt[:6]
                        + cc0
                        + nxt[6:]
                        + [None] * (ogs - len(nxt) - len(cc0))
                        + [lambda: og_dma(0, nc.sync)]
                        + [None] * (5 if et else 9)
                        + [lambda u=u: proj_u(0, u) for u in range(4)]
                        + ([None] * 10 if et else [])
                    )
                else:
                    fillers_b1 = (
                        [None] * 6
                        + cc0
                        + [None] * (44 - len(cc0))
                        + [lambda: og_dma(0, nc.sync)]
                        + [None] * 9
                        + [lambda u=u: proj_u(0, u) for u in range(4)]
                    )
                rounds(1, fillers_b1)
                if flow:
                    while pend:
                        flush_one()
                emit_cc(1)
                if fastpro >= 3:
                    # SP is clean here (next pass's xt loads already issued
                    # inside rounds(1)), so og(1) can ride SP without
                    # head-of-line blocking anything urgent
                    og_dma(1, nc.sync)
                else:
                    # ACT hwdge queue: the next pass's xt loads on SP must
                    # not head-of-line block behind this A2A-dependent DMA
                    og_dma(1, nc.scalar)

                if last:
                    # keep the PE clock warm through the tail A2A window so
                    # proj-b1 doesn't run at the cold p-state
                    for w in range(warmup):
                        psw = fl_alloc()
                        for c in range(ND):
                            nc.tensor.matmul(
                                psw[:],
                                og_s[:, 0, c, 0:128],
                                wp_s[:, c, 0:512],
                                start=(c == 0),
                                stop=(c == ND - 1),
                            )
                else:
                    # next pass's prologue fills the tail A2A window with
                    # real work (and keeps the PE clock warm)
                    prologue(first=False)
                if (pj1f or fastpro >= 3) and not last:
                    # og(1) is ready well before the next rounds-b0 begins;
                    # running proj-b1 as its early fillers removes ~7us of
                    # serial tail per pass
                    deferred_proj[0] = [lambda u=u: proj_u(1, u) for u in range(4)]
                else:
                    for u in range(4):
                        proj_u(1, u)

            prologue(first=True)
            for rep in range(repeat):
                one_body(last=(rep == repeat - 1) or bare)
                if bare and rep + 1 < repeat:
                    prologue(first=False)

    nc.compile()
    return nc


def _prep_inputs(x, W_qkv, W_proj, b_proj):
    """Host-side sharding: per-core input dicts."""
    import ml_dtypes

    bf16 = np.dtype(ml_dtypes.bfloat16)
    x = np.asarray(x, dtype=np.float32)
    W_qkv = np.asarray(W_qkv, dtype=np.float32)
    W_proj = np.asarray(W_proj, dtype=np.float32)
    b_proj = np.asarray(b_proj, dtype=np.float32)

    # xt [B, DIM, P] bf16 — identical on every core
    xt = np.ascontiguousarray(x.transpose(0, 2, 1)).astype(bf16)
    # wp row 128s + 64p + i = W_proj[64*(2s+p) + i] — exactly natural order
    wp = np.ascontiguousarray(W_proj).astype(bf16)
    bias_b = np.ascontiguousarray(np.broadcast_to(b_proj[None, :], (128, DIM)))

    in_maps = []
    for c in range(N_CORES):
        h0 = 2 * c
        cols = slice(D * h0, D * (h0 + 2))
        in_maps.append(
            {
                "xt": xt,
                "wq": np.ascontiguousarray(W_qkv[:, 0 * DIM :][:, cols]).astype(bf16),
                "wk": np.ascontiguousarray(W_qkv[:, 1 * DIM :][:, cols]).astype(bf16),
                "wv": np.ascontiguousarray(W_qkv[:, 2 * DIM :][:, cols]).astype(bf16),
                "wp": wp,
                "bias": bias_b,
            }
        )
    return in_maps


def kernel(x, W_qkv, W_proj, b_proj, _trace=False, _tmpdir=None):
    if "nc" not in _CACHE:
        _CACHE["nc"] = _build()
    nc = _CACHE["nc"]
    in_maps = _prep_inputs(x, W_qkv, W_proj, b_proj)
    res = run_bass_kernel_spmd(
        nc,
        in_maps,
        core_ids=list(range(N_CORES)),
        trace=_trace,
        tmpdir=_tmpdir,
        stitch_traces=False,
    )
    _CACHE["last_results"] = res
    full = np.empty((B, P, DIM), dtype=np.float32)
    for c in range(N_CORES):
        o = np.asarray(res.results[c]["out"])
        for b in range(B):
            full[b, QS * c : QS * (c + 1), :] = o[b]
    return full

